# revision 37
# baseline (speedup 1.0000x reference)
"""Multi-head channel-attention kernel for Trainium2 (8 NeuronCores, SPMD).

Reference computation (per batch b, x = [256, N] with N = 64*64 = 4096):
    qkv   = w_qkv @ x
    q,k,v = per-head [256, N] slices of qkv
    logit = (q*scale) @ k.T          # [256, 256] (contraction over N)
    wts   = softmax(logit, -1)
    out_h = wts @ v
    y     = w_out @ stack_h(out_h) + b_out

Distribution: pure data-parallel — batch 8 across 8 cores, one batch per
core, no collectives.

The kernel exploits that attention is over the *channel* axis (n >> c):

    logit_h = (Wq_h * scale) @ (x @ x.T) @ Wk_h.T
    y       = (sum_h W_h @ softmax_h @ Wv_h) @ x + b  =  Wstar @ x + b

so the only n-wide work is the Gram matrix G = x @ x.T (one pass over x)
and the final Wstar @ x (second pass). Everything else is [256,256]-sized.
Per-batch FLOPs drop from 12.9G (direct) to 1.6G.

Pipeline (all matmuls TensorE, bf16 operands, fp32 PSUM):
    G    = xT.T @ xT                  (xT shipped pre-transposed from host)
    A_h  = G @ Wk_h.T                 (uses G's symmetry: lhsT = G)
    L_h  = (Wq_h*scale) @ A_h         -> PSUM
    E_h  = exp(L_h) on ScalarE straight from PSUM, accum_out = row sums;
           row-normalize with VectorE reciprocal (softmax; logits are O(1)
           for this problem so no max-subtraction is needed)
    M_hT = E_h-contraction with WoT   (computed directly transposed:
           lhsT = Ehat, rhs = WoT — no on-chip transposes anywhere)
    WstarT += Wv_h-contraction with M_hT  (accumulated head-by-head in the
           G PSUM banks, interleaved into the pipeline two stages behind
           M_h so it never waits on M's SBUF drain)
    y    = WstarT.T @ x + b           (bias folded into the PSUM drains,
           which alternate VectorE/ScalarE; fp16 output halves the 4MB
           store traffic, shipped as [128,1024] j-pairs — 2KB DMA rows —
           with trigger chains split across the Sync and GpSimd queues)

PE emission order: G, then A0 A1 L0 A2 L1 A3 L2 M0 L3 M1 W0 M2 W1 M3 W2 W3
— each M_h trails its softmax chain by >=3 PE stages and each W_h trails
M_h's drain by >=2, so the PE stays fed through the softmax latency.

Weights are repacked/pre-transposed on the host; x is shipped twice
(native and transposed, bf16) so the Gram matmul needs no on-chip
transpose. The Scalar-queue x/b triggers are emitted after the last exp:
ACT executes its DMA-trigger instructions inline and ~1us of
flow-controlled trigger ahead of the first exp stalls the softmax chain.
"""

import numpy as np
import ml_dtypes

import concourse.bass as bass
import concourse.mybir as mybir
import concourse.tile as tile
from concourse.bass import ts
from concourse.bass_utils import run_bass_kernel_spmd
from concourse.vector_clock import ScopedClock

B, DIM, H, W = 8, 256, 64, 64
HEADS = 4
N = H * W            # 4096
P = 128
KT = DIM // P        # 2 channel tiles
NT = N // P          # 32 n-tiles of 128
NQ = 4               # xT shipped in 4 slabs of 8 n-tiles (2KB fp8 rows)
NCH = N // 512       # 8 n-chunks of 512
N_CORES = 8

F32 = mybir.dt.float32
F16 = mybir.dt.float16
BF16 = mybir.dt.bfloat16
F8 = mybir.dt.float8e4
NPBF16 = ml_dtypes.bfloat16
NPF8 = ml_dtypes.float8_e4m3fn if hasattr(ml_dtypes, "float8_e4m3fn") else ml_dtypes.float8_e4m3
# xt is shipped fp8(e4m3), x scaled by 1/XT_SCALE so G's diagonal
# (~4096) fits the e4m3 range; the 1/XT_SCALE^2 on G is folded into the
# host-side wq scaling. Costs ~6e-3 extra rel err (measured 1.3e-2
# total vs the 2e-2 gate), halves the xt DMA and doubles G's
# arrival rate.
XT_SCALE = 8.0


def _split_multi_waits(nc, max_waits=1):
    """The walrus build in this container rejects instructions carrying more
    than one sync-wait. Move excess waits onto same-engine carrier NOPs
    inserted immediately before the instruction (engines are in-order, so
    waiting earlier on the same stream is equivalent)."""
    n_split = 0
    for f in nc.m.functions:
        for bb in f.blocks:
            old = list(bb.instructions)
            new = []
            changed = False
            for inst in old:
                si = inst.sync_info
                waits = list(si.on_wait) if si and si.on_wait else []
                if len(waits) > max_waits:
                    changed = True
                    # reversed: the late-retiring semaphores (DMA-queue
                    # completions) sit early in the tile framework's wait
                    # lists, so keeping the original order serializes ~8
                    # carrier NOPs AFTER the last gate retires; reversed,
                    # the early-retiring engine sems drain the NOP chain
                    # first (pure reorder — every wait is still enforced)
                    for w in reversed(waits[max_waits:]):
                        n_split += 1
                        new.append(
                            mybir.InstNoOp(
                                name=f"wsplit_{n_split}_{inst.name}",
                                engine=inst.engine,
                                ins=[],
                                outs=[],
                                sync_info=mybir.SyncInfo(on_wait=[w], on_update=[]),
                            )
                        )
                    inst.sync_info = mybir.SyncInfo(
                        on_wait=waits[:max_waits], on_update=si.on_update
                    )
                new.append(inst)
            if changed:
                bb.instructions = new
    return n_split


def _minimal_exit(self, tick_clock, wait_clock):
    """TileContext._drain_and_barrier replacement: one SP drain carrying the
    global-clock waits (split onto NOPs by _split_multi_waits afterwards).

    The stock exit adds two all-engine barriers and ~200 per-semaphore
    clears (~10 us). They are redundant here: the bass preamble range-clears
    the whole kernel semaphore range at startup, and bass's own postamble
    still drains every engine.
    """
    nc = self.nc
    drain = nc.sync.drain()
    wait_clock.add_sem_waits(drain.ins, ScopedClock({None: tick_clock.global_clock}))
    popped = nc._tile_sem_poison_stack.pop()
    assert popped is self._sem_poison


def build_program():
    """Build the single-core Bass program (run SPMD across 8 cores)."""
    nc = bass.Bass()

    x_d = nc.declare_dram_parameter("x", [DIM, N], BF16, isOutput=False)
    # xt: [NQ][128, 8, 256] fp8; slab qi, element (p, a, c) = x.T[qi*1024 + a*128 + p, c] / XT_SCALE
    xt_d = nc.declare_dram_parameter("xt", [NQ, P, NT // NQ, DIM], F8, isOutput=False)
    # wkq: [KT][128, 2048+128] = [wqT | wkT | I128 (k=0) / 0 (k=1)];
    # wvo: [KT][128, 2048] = [wv | woT]. The 128x128 identity rides the
    # wkq load (used by the PE-mode transpose reconstructing G's (1,0)
    # block) — shipping it host-side avoids a slow GpSimd affine_select.
    wkq_d = nc.declare_dram_parameter("wkq", [KT, P, 2 * HEADS * DIM + P], BF16, isOutput=False)
    wvo_d = nc.declare_dram_parameter("wvo", [KT, P, 2 * HEADS * DIM], BF16, isOutput=False)
    b_d = nc.declare_dram_parameter("b", [DIM, 1], F32, isOutput=False)
    # y stored fp16: |y| <= ~0.4 here, fp16 adds ~5e-4 rel error but halves
    # the 4MB output DMA that dominated the kernel tail.
    y_d = nc.declare_dram_parameter("y", [DIM, N], F16, isOutput=True)

    OQT, OKT, OV, OOT = 0, HEADS * DIM, 0, HEADS * DIM

    prev_exit = tile.TileContext._drain_and_barrier
    tile.TileContext._drain_and_barrier = _minimal_exit
    try:
        _build_body(nc, tc_args=(x_d, xt_d, wkq_d, wvo_d, b_d, y_d, OQT, OKT, OV, OOT))
    finally:
        tile.TileContext._drain_and_barrier = prev_exit

    # NOTE: hoisting startup work before the init barrier was tried and lost
    # time — the runtime preamble (~6.5us) gates all engines anyway, and
    # pre-barrier work just delays the barrier release for everyone.
    _split_multi_waits(nc)
    return nc


def _build_body(nc, tc_args):
    x_d, xt_d, wkq_d, wvo_d, b_d, y_d, OQT, OKT, OV, OOT = tc_args
    with tile.TileContext(nc) as tc:
        with (
            tc.tile_pool(name="wpool", bufs=1) as wpool,
            tc.tile_pool(name="spool", bufs=2) as spool,
            tc.tile_pool(name="ypool", bufs=2) as ypool,
            tc.tile_pool(name="psum", bufs=1, space="PSUM") as psum,
        ):
            # ---- PE warmup: dummy matmuls during the input DMAs release
            # the HAM clock-gate; G's first xT slab completes ~10.3us
            # (per-queue bandwidth bound), so the warmup just covers the
            # wait.
            warm = wpool.tile([P, P], BF16, tag="warm")
            nc.gpsimd.memset(warm[:], 0)
            # dummy Pool tensor_scalar: forces the GPSIMD ucode library
            # load during the input DMAs, not at the first softmax scale
            warms = wpool.tile([P, 1], F32, tag="warms")
            nc.gpsimd.memset(warms[:], 0)
            nc.gpsimd.tensor_scalar_mul(warm[:, 0:1], warm[:, 0:1], warms[:])
            # warmup accumulates in the M banks (idle until ~22us) so G's
            # first matmul never waits on warmup retirement in its own bank.
            # 30 dummies cover the PE from the barrier (~7.8us) until xt
            # slab0 lands (~10.5us) with no idle gap, so the HAM clock-gate
            # flips to 8/8 at ~11us and G runs warm (2.4GHz) nearly
            # throughout; any idle gap resets the 3.4us HAM busy-window.
            wps = psum.tile([P, P], F32, tag="m", bufs=2)
            for _ in range(34):
                nc.tensor.matmul(wps[:], warm[:], warm[:], start=True, stop=True)

            # ---- loads (xT slabs first: G consumes them incrementally;
            # triggers split across the two HWDGE engines (SP + ACT) so the
            # trigger chains run in parallel) ----
            # xT slabs alternate the two HWDGE queues (Sync/Scalar).
            # NOTE: adding the GpSimd queue as a third input lane was tried
            # and lost time — qGpSimdDynamic moves bulk data at only
            # ~86 B/ns (half a HWDGE queue), so its slabs arrive LATE and
            # stall G. It is fine for the small y-phase store triggers.
            # NOTE: half-splitting slab0 for an earlier G start was tried —
            # the halves use 1KB DMA rows at ~half throughput, stretching
            # slab0 to ~3us, stalling G mid-stream and delaying every
            # later Sync-queue slab. Full 2KB-row slabs win.
            xt_sb = []
            for qi in range(NQ):
                t = wpool.tile([P, NT // NQ, DIM], F8, tag=f"xt{qi}")
                eng = nc.sync if qi % 2 == 0 else nc.scalar
                eng.dma_start(t[:], xt_d[qi])
                xt_sb.append(t)
            wkq_sb = []
            for k in range(KT):
                t = wpool.tile([P, 2 * HEADS * DIM + P], BF16, tag=f"wkq{k}")
                eng = nc.sync if k == 0 else nc.scalar
                eng.dma_start(t[:], wkq_d[k])
                wkq_sb.append(t)
            ident = wkq_sb[0][:, 2 * HEADS * DIM : 2 * HEADS * DIM + P]
            wvo_sb = []
            for k in range(KT):
                t = wpool.tile([P, 2 * HEADS * DIM], BF16, tag=f"wvo{k}")
                eng = nc.sync if k == 0 else nc.scalar
                eng.dma_start(t[:], wvo_d[k])
                wvo_sb.append(t)
            # dummy ACT op: forces the one-time ~1.3us activation-table
            # preload during the input DMAs instead of at the first real ACT
            # copy on the G->A critical path. Emitted AFTER the Scalar-queue
            # input triggers: ACT runs its trigger instructions inline, and
            # putting the table load first delayed Q10's first data ~1.3us.
            warm2 = wpool.tile([P, 1], BF16, tag="warm2")
            nc.scalar.copy(warm2[:], warm[:, 0:1])
            # x is shipped in [128, 2048] chunks (4KB DMA rows, full rate),
            # k0 chunks on the Sync ring, k1 chunks on the Scalar ring,
            # AFTER the xt/weight stream: the y GEMM consumes chunk c as
            # soon as both k-halves land instead of waiting for the whole
            # 2MB of x, and x never competes with the xt stream that paces
            # G. All triggers are emitted at load time: a trigger costs
            # ~0.6us of issuing-engine time, and emitting them late lets
            # the tile scheduler interleave them into ACT's softmax chain.
            XC = 2               # 2 chunks of 2048 per k-tile
            x_ch = [[wpool.tile([P, N // XC], BF16, tag=f"x{k}_{c}",
                                name=f"xsb{k}_{c}") for c in range(XC)]
                    for k in range(KT)]
            b_sb = []
            for ot in range(KT):
                b_sb.append(wpool.tile([P, 1], F32, tag=f"b{ot}", name=f"bsb{ot}"))
            # both k rows of x ride the Sync ring, interleaved so chunk
            # PAIRS complete together. Issuing any x triggers from ACT is
            # poison: the tile scheduler slots the flow-controlled ~0.6us
            # trigger instructions between ACT's softmax-chain ops.
            for c in range(XC):
                nc.sync.dma_start(x_ch[0][c][:], x_d[ts(0, P), ts(c, N // XC)])
                nc.sync.dma_start(x_ch[1][c][:], x_d[ts(1, P), ts(c, N // XC)])
            nc.sync.dma_start(b_sb[0][:], b_d[ts(0, P), :])
            nc.sync.dma_start(b_sb[1][:], b_d[ts(1, P), :])

            # ---- G = x @ x.T (fp32 PSUM, 32 accumulation steps) ----------
            # G is symmetric: compute row-tile 0 in full ([128, 256]) and
            # only the diagonal block of row-tile 1 ([128, 128], cols
            # 128:256); the (1,0) block is the PE-transposed (0,1) block.
            # Saves 32 N=128 matmuls (~1.7us warm) for one transpose+drain.
            g_ps = []
            for ct in range(KT):
                gp = psum.tile([P, DIM], F32, tag=f"g{ct}", bufs=1)
                g_ps.append(gp)
            for i in range(NT):
                qi, a = divmod(i, NT // NQ)
                nc.tensor.matmul(
                    g_ps[0][:],
                    xt_sb[qi][:, a, ts(0, P)],
                    xt_sb[qi][:, a, :],
                    start=(i == 0),
                    stop=(i == NT - 1),
                )
                nc.tensor.matmul(
                    g_ps[1][:, ts(1, P)],
                    xt_sb[qi][:, a, ts(1, P)],
                    xt_sb[qi][:, a, ts(1, P)],
                    start=(i == 0),
                    stop=(i == NT - 1),
                )
            g_sb = []
            for ct in range(KT):
                g = spool.tile([P, DIM], BF16, tag=f"gs{ct}", bufs=1, name=f"g{ct}")
                g_sb.append(g)
            # (0,1) block drains first on DVE: the transpose consumes it
            nc.vector.tensor_copy(g_sb[0][:, ts(1, P)], g_ps[0][:, ts(1, P)])
            nc.scalar.copy(g_sb[0][:, ts(0, P)], g_ps[0][:, ts(0, P)])
            nc.scalar.copy(g_sb[1][:, ts(1, P)], g_ps[1][:, ts(1, P)])
            # transpose lands in the first 'a'-rotation bank: its DVE drain
            # completes well before stage_A's second ct reuses that bank
            gtp = psum.tile([P, P], BF16, tag="a", bufs=2, name="gtp")
            nc.tensor.transpose(gtp[:], g_sb[0][:, ts(1, P)], ident)
            nc.vector.tensor_copy(g_sb[1][:, ts(0, P)], gtp[:])

            # ---- per-head stages, software-pipelined across heads --------
            # stage A(h): A = G @ Wk_h.T          (PE + drain)
            # stage L(h): L = (Wq_h*scale) @ A    (PE -> PSUM) + softmax
            # stage M(h): M_hT = Ehat . WoT       (PE + drain)
            # stage W(h): WstarT += Wv_h . M_hT   (PSUM accum in the G banks)
            a_all, es_all, lp_all = {}, {}, {}
            m_sb = {}
            wp_all = [
                psum.tile([P, DIM], F32, tag=f"g{ct}", bufs=1, name=f"wp{ct}")
                for ct in range(KT)
            ]
            wst_sb = []

            def stage_A(h):
                a_sb = []
                for ct in range(KT):
                    ap = psum.tile([P, DIM], F32, tag="a", bufs=2, name=f"ap{h}_{ct}")
                    for k in range(KT):
                        # A[c', d] = sum_c'' G[c'', c'] wkT[c'', d]  (G symmetric)
                        nc.tensor.matmul(
                            ap[:],
                            g_sb[k][:, ts(ct, P)],
                            wkq_sb[k][:, OKT + h * DIM : OKT + (h + 1) * DIM],
                            start=(k == 0),
                            stop=(k == KT - 1),
                        )
                    at = spool.tile([P, DIM], BF16, tag=f"a{ct}", name=f"at{h}_{ct}")
                    # all PSUM drains go to DVE: ACT is the head-phase
                    # pacer (exps + accumulator reads ~6.2us), while the
                    # softmax scale now runs on the idle Pool engine
                    nc.vector.tensor_copy(at[:], ap[:])
                    a_sb.append(at)
                a_all[h] = a_sb

            def stage_L(h):
                pl = []
                for ct in range(KT):
                    lp = psum.tile([P, DIM], F32, tag=f"l{ct}", bufs=1, name=f"lp{h}_{ct}")
                    for k in range(KT):
                        # L[c, d] = sum_c' wqT[c', c] A[c', d]
                        nc.tensor.matmul(
                            lp[:],
                            wkq_sb[k][:, OQT + h * DIM + ct * P : OQT + h * DIM + (ct + 1) * P],
                            a_all[h][k][:],
                            start=(k == 0),
                            stop=(k == KT - 1),
                        )
                    pl.append(lp)
                lp_all[h] = pl
                # softmax immediately (ACT/DVE; doesn't occupy the PE)
                es = []
                for ct in range(KT):
                    e = spool.tile([P, DIM], BF16, tag=f"e{ct}", name=f"e{h}_{ct}")
                    s = spool.tile([P, 1], F32, tag=f"s{ct}", name=f"s{h}_{ct}")
                    r = spool.tile([P, 1], F32, tag=f"r{ct}", name=f"r{h}_{ct}")
                    nc.scalar.activation(
                        e[:], pl[ct][:], mybir.ActivationFunctionType.Exp,
                        accum_out=s[:],
                    )
                    nc.vector.reciprocal(r[:], s[:])
                    # softmax scaling on Pool: SBUF-only op, Pool is idle,
                    # and it takes ~290ns each off the ACT/DVE drain budget
                    nc.gpsimd.tensor_scalar_mul(e[:], e[:], r[:])
                    es.append(e)
                es_all[h] = es

            def stage_M(h):
                es = es_all[h]
                for dt2 in range(KT):
                    pm = psum.tile([P, DIM], F32, tag="m", bufs=2, name=f"pm{h}_{dt2}")
                    for ct in range(KT):
                        # M_hT[d, o] = sum_c Ehat[c, d] woT[c, o]
                        nc.tensor.matmul(
                            pm[:],
                            es[ct][:, ts(dt2, P)],
                            wvo_sb[ct][:, OOT + h * DIM : OOT + (h + 1) * DIM],
                            start=(ct == 0),
                            stop=(ct == KT - 1),
                        )
                    mt = spool.tile([P, DIM], BF16, tag=f"m{h}_{dt2}", bufs=1,
                                    name=f"mt{h}_{dt2}")
                    m_sb[(h, dt2)] = mt
                    nc.vector.tensor_copy(mt[:], pm[:])

            def stage_W(h):
                # WstarT[c_in, o] += sum_d wv[d, c_in] M_hT[d, o]
                for ct in range(KT):
                    for dt2 in range(KT):
                        nc.tensor.matmul(
                            wp_all[ct][:],
                            wvo_sb[dt2][:, OV + h * DIM + ct * P : OV + h * DIM + (ct + 1) * P],
                            m_sb[(h, dt2)][:],
                            start=(h == 0 and dt2 == 0),
                            stop=(h == HEADS - 1 and dt2 == KT - 1),
                        )
                if h == HEADS - 1:
                    for ct in range(KT):
                        wt = spool.tile([P, DIM], BF16, tag=f"wst{ct}", bufs=1,
                                        name=f"wt{ct}")
                        # split by column half: the first y matmuls read
                        # only the ot0 halves, so draining those first
                        # starts the y phase earlier while the ot1 halves
                        # drain under the y GEMM. ct0 on DVE, ct1 on ACT
                        # (ACT's exps are done by now).
                        for half in range(KT):
                            src = wp_all[ct][:, ts(half, P)]
                            if ct == 0:
                                nc.vector.tensor_copy(wt[:, ts(half, P)], src)
                            else:
                                nc.scalar.copy(wt[:, ts(half, P)], src)
                        wst_sb.append(wt)

            # pipelined emission:
            # PE order A0 A1 L0 A2 L1 A3 L2 M0 L3 M1 W0 M2 W1 M3 W2 W3
            stage_A(0)
            stage_A(1)
            stage_L(0)
            stage_A(2)
            stage_L(1)
            stage_A(3)
            stage_L(2)
            stage_M(0)
            stage_L(3)
            stage_M(1)
            stage_W(0)
            stage_M(2)
            stage_W(1)
            stage_M(3)
            stage_W(2)
            stage_W(3)

            # ---- y = WstarT.T @ x + b ------------------------------------
            # drains alternate DVE/ACT writing fp16; chunks are DMA'd in
            # [128, 1024] j-pairs (2KB rows for full DMA descriptor
            # efficiency), trigger chains split across Sync/GpSimd queues
            y_sb = {}
            for ot in range(KT):
                y_sb[ot] = ypool.tile([P, N], F16, tag=f"y{ot}", bufs=1,
                                      name=f"ysb{ot}")
            ycnt = 0
            # 6-bank rotation: with only 4 banks the j+2'th matmul waits on
            # the j'th chunk's ~520ns drain, pacing the y GEMM at ~275ns/MM
            # instead of the 213ns warm matmul rate. The g banks are idle
            # once Wstar has drained, so they join the rotation (bufs=1
            # matches their earlier allocation).
            ytags = ["a", "a", "m", "m", "g0", "g1"]
            ybufs = {"a": 2, "m": 2, "g0": 1, "g1": 1}
            for j in range(NCH):
                for ot in range(KT):
                    ytag = ytags[ycnt % 6]
                    py = psum.tile([P, 512], F32, tag=ytag, bufs=ybufs[ytag],
                                   name=f"py{j}_{ot}")
                    for k in range(KT):
                        nc.tensor.matmul(
                            py[:],
                            wst_sb[k][:, ts(ot, P)],
                            x_ch[k][j // 4][:, ts(j % 4, 512)],
                            start=(k == 0),
                            stop=(k == KT - 1),
                        )
                    dst = y_sb[ot][:, ts(j, 512)]
                    if ycnt % 2 == 0:
                        nc.vector.tensor_scalar_add(dst, py[:], b_sb[ot][:])
                    else:
                        nc.scalar.add(dst, py[:], b_sb[ot][:])
                    ycnt += 1
                    # uniform j-pair stores: 2KB DMA rows run at ~2x the
                    # rate of 1KB-row singles, so a final 256KB pair beats
                    # a tapered 128KB single. All stores ride the Sync
                    # HWDGE ring (~300 B/ns solo, SP engine idle by now);
                    # the SWDGE ring only reached ~150 B/ns and its last
                    # transfer trailed the final matmul by ~4.6us. ACT as
                    # a lane was tried and lost 4us (trigger instructions
                    # land between ACT's y drains).
                    if j % 2 == 1:
                        nc.sync.dma_start(
                            y_d[ts(ot, P), (j - 1) * 512 : (j + 1) * 512],
                            y_sb[ot][:, (j - 1) * 512 : (j + 1) * 512],
                        )


def prep_inputs(x, w_qkv, w_out, b_out):
    """Host-side packing: per-core input dicts (numpy only)."""
    x = np.asarray(x, dtype=np.float32)
    w_qkv = np.asarray(w_qkv, dtype=np.float32)
    w_out = np.asarray(w_out, dtype=np.float32)
    b_out = np.asarray(b_out, dtype=np.float32)

    scale = float(DIM) ** -0.5
    wq = w_qkv[0 * HEADS * DIM : 1 * HEADS * DIM].reshape(HEADS, DIM, DIM)
    wk = w_qkv[1 * HEADS * DIM : 2 * HEADS * DIM].reshape(HEADS, DIM, DIM)
    wv = w_qkv[2 * HEADS * DIM : 3 * HEADS * DIM].reshape(HEADS, DIM, DIM)

    # wqT[c', h*256 + c] = wq[h, c, c'] * scale * XT_SCALE^2 (undoes the
    # fp8 xt downscale baked into G)
    wqT = (np.transpose(wq, (2, 0, 1)) * (scale * XT_SCALE * XT_SCALE)).reshape(
        DIM, HEADS * DIM)
    # wkT[c', h*256 + d] = wk[h, d, c']
    wkT = np.transpose(wk, (2, 0, 1)).reshape(DIM, HEADS * DIM)
    # wvn[d, h*256 + c_in] = wv[h, d, c_in]  (natural orientation, head-concat)
    wvn = np.transpose(wv, (1, 0, 2)).reshape(DIM, HEADS * DIM)
    # woT[c, h*256 + o] = w_out[o, c*HEADS + h]
    woT = np.ascontiguousarray(
        w_out.reshape(DIM, DIM, HEADS).transpose(1, 2, 0)
    ).reshape(DIM, HEADS * DIM)

    # wkq[k] = [wqT | wkT | I/0], wvo[k] = [wv | woT], rows k*128:(k+1)*128
    id_col = np.concatenate([np.eye(P, dtype=np.float32),
                             np.zeros((P, P), dtype=np.float32)], axis=0)
    wkq = np.ascontiguousarray(
        np.concatenate([wqT, wkT, id_col], axis=1).astype(NPBF16)
        .reshape(KT, P, 2 * HEADS * DIM + P)
    )
    wvo = np.ascontiguousarray(
        np.concatenate([wvn, woT], axis=1).astype(NPBF16).reshape(KT, P, 2 * HEADS * DIM)
    )
    b = b_out.reshape(DIM, 1).astype(np.float32)

    in_maps = []
    for bi in range(B):
        xbf = np.ascontiguousarray(x[bi].reshape(DIM, N))
        xb = xbf.astype(NPBF16)
        # xt[qi, p, a, c] = x.T[qi*(N//NQ) + a*128 + p, c] / XT_SCALE, fp8
        xt = np.ascontiguousarray(
            (xbf.T / XT_SCALE).astype(NPF8).reshape(NQ, NT // NQ, P, DIM)
            .transpose(0, 2, 1, 3)
        )
        in_maps.append({"x": xb, "xt": xt, "wkq": wkq, "wvo": wvo, "b": b})
    return in_maps


_NC_CACHE = {}


def get_program():
    if "nc" not in _NC_CACHE:
        _NC_CACHE["nc"] = build_program()
    return _NC_CACHE["nc"]


def kernel(x, w_qkv, w_out, b_out, **_unused):
    nc = get_program()
    in_maps = prep_inputs(x, w_qkv, w_out, b_out)
    res = run_bass_kernel_spmd(nc, in_maps, list(range(N_CORES)))
    y = np.stack([res.results[c]["y"] for c in range(N_CORES)], axis=0)
    return y.reshape(B, DIM, H, W).astype(np.float32)



# revision 39
# speedup vs baseline: 1.7059x; 1.7059x over previous
"""Multi-head channel-attention kernel for Trainium2 (8 NeuronCores, SPMD).

Reference computation (per batch b, x = [256, N] with N = 64*64 = 4096):
    qkv   = w_qkv @ x
    q,k,v = per-head [256, N] slices of qkv
    logit = (q*scale) @ k.T          # [256, 256] (contraction over N)
    wts   = softmax(logit, -1)
    out_h = wts @ v
    y     = w_out @ stack_h(out_h) + b_out

Distribution: pure data-parallel — batch 8 across 8 cores, one batch per
core, no collectives.

The kernel exploits that attention is over the *channel* axis (n >> c):

    logit_h = (Wq_h * scale) @ (x @ x.T) @ Wk_h.T
    y       = (sum_h W_h @ softmax_h @ Wv_h) @ x + b  =  Wstar @ x + b

so the only n-wide work is the Gram matrix G = x @ x.T (one pass over x)
and the final Wstar @ x (second pass). Everything else is [256,256]-sized.
Per-batch FLOPs drop from 12.9G (direct) to 1.6G.

Pipeline (all matmuls TensorE, bf16 operands, fp32 PSUM):
    G    = xT.T @ xT                  (xT shipped pre-transposed from host)
    A_h  = G @ Wk_h.T                 (uses G's symmetry: lhsT = G)
    L_h  = (Wq_h*scale) @ A_h         -> PSUM
    E_h  = exp(L_h) on ScalarE straight from PSUM, accum_out = row sums;
           row-normalize with VectorE reciprocal (softmax; logits are O(1)
           for this problem so no max-subtraction is needed)
    M_hT = E_h-contraction with WoT   (computed directly transposed:
           lhsT = Ehat, rhs = WoT — no on-chip transposes anywhere)
    WstarT += Wv_h-contraction with M_hT  (accumulated head-by-head in the
           G PSUM banks, interleaved into the pipeline two stages behind
           M_h so it never waits on M's SBUF drain)
    y    = WstarT.T @ x + b           (bias folded into the PSUM drains,
           which alternate VectorE/ScalarE; fp16 output halves the 4MB
           store traffic, shipped as [128,1024] j-pairs — 2KB DMA rows —
           with trigger chains split across the Sync and GpSimd queues)

PE emission order: G, then A0 A1 L0 A2 L1 A3 L2 M0 L3 M1 W0 M2 W1 M3 W2 W3
— each M_h trails its softmax chain by >=3 PE stages and each W_h trails
M_h's drain by >=2, so the PE stays fed through the softmax latency.

Weights are repacked/pre-transposed on the host; x is shipped twice
(native and transposed, bf16) so the Gram matmul needs no on-chip
transpose. The Scalar-queue x/b triggers are emitted after the last exp:
ACT executes its DMA-trigger instructions inline and ~1us of
flow-controlled trigger ahead of the first exp stalls the softmax chain.
"""

import numpy as np
import ml_dtypes

import concourse.bass as bass
import concourse.mybir as mybir
import concourse.tile as tile
from concourse.bass import ts
from concourse.bass_utils import run_bass_kernel_spmd
from concourse.vector_clock import ScopedClock

B, DIM, H, W = 8, 256, 64, 64
HEADS = 4
N = H * W            # 4096
P = 128
KT = DIM // P        # 2 channel tiles
NT = N // P          # 32 n-tiles of 128
NQ = 4               # xT shipped in 4 slabs of 8 n-tiles (2KB fp8 rows)
NCH = N // 512       # 8 n-chunks of 512
N_CORES = 8

F32 = mybir.dt.float32
F16 = mybir.dt.float16
BF16 = mybir.dt.bfloat16
F8 = mybir.dt.float8e4
NPBF16 = ml_dtypes.bfloat16
NPF8 = ml_dtypes.float8_e4m3fn if hasattr(ml_dtypes, "float8_e4m3fn") else ml_dtypes.float8_e4m3
# xt is shipped fp8(e4m3), x scaled by 1/XT_SCALE so G's diagonal
# (~4096) fits the e4m3 range; the 1/XT_SCALE^2 on G is folded into the
# host-side wq scaling. Costs ~6e-3 extra rel err (measured 1.3e-2
# total vs the 2e-2 gate), halves the xt DMA and doubles G's
# arrival rate.
XT_SCALE = 8.0


def _split_multi_waits(nc, max_waits=1):
    """The walrus build in this container rejects instructions carrying more
    than one sync-wait. Move excess waits onto same-engine carrier NOPs
    inserted immediately before the instruction (engines are in-order, so
    waiting earlier on the same stream is equivalent)."""
    n_split = 0
    for f in nc.m.functions:
        for bb in f.blocks:
            old = list(bb.instructions)
            new = []
            changed = False
            for inst in old:
                si = inst.sync_info
                waits = list(si.on_wait) if si and si.on_wait else []
                if len(waits) > max_waits:
                    changed = True
                    # reversed: the late-retiring semaphores (DMA-queue
                    # completions) sit early in the tile framework's wait
                    # lists, so keeping the original order serializes ~8
                    # carrier NOPs AFTER the last gate retires; reversed,
                    # the early-retiring engine sems drain the NOP chain
                    # first (pure reorder — every wait is still enforced)
                    for w in reversed(waits[max_waits:]):
                        n_split += 1
                        new.append(
                            mybir.InstNoOp(
                                name=f"wsplit_{n_split}_{inst.name}",
                                engine=inst.engine,
                                ins=[],
                                outs=[],
                                sync_info=mybir.SyncInfo(on_wait=[w], on_update=[]),
                            )
                        )
                    inst.sync_info = mybir.SyncInfo(
                        on_wait=waits[:max_waits], on_update=si.on_update
                    )
                new.append(inst)
            if changed:
                bb.instructions = new
    return n_split


def _minimal_exit(self, tick_clock, wait_clock):
    """TileContext._drain_and_barrier replacement: one SP drain carrying the
    global-clock waits (split onto NOPs by _split_multi_waits afterwards).

    The stock exit adds two all-engine barriers and ~200 per-semaphore
    clears (~10 us). They are redundant here: the bass preamble range-clears
    the whole kernel semaphore range at startup, and bass's own postamble
    still drains every engine.
    """
    nc = self.nc
    drain = nc.sync.drain()
    wait_clock.add_sem_waits(drain.ins, ScopedClock({None: tick_clock.global_clock}))
    popped = nc._tile_sem_poison_stack.pop()
    assert popped is self._sem_poison


def build_program():
    """Build the single-core Bass program (run SPMD across 8 cores)."""
    nc = bass.Bass()

    x_d = nc.declare_dram_parameter("x", [DIM, N], BF16, isOutput=False)
    # xt: [NQ][128, 8, 256] fp8; slab qi, element (p, a, c) = x.T[qi*1024 + a*128 + p, c] / XT_SCALE
    xt_d = nc.declare_dram_parameter("xt", [NQ, P, NT // NQ, DIM], F8, isOutput=False)
    # wkq: [KT][128, 2048+128] = [wqT | wkT | I128 (k=0) / 0 (k=1)];
    # wvo: [KT][128, 2048] = [wv | woT]. The 128x128 identity rides the
    # wkq load (used by the PE-mode transpose reconstructing G's (1,0)
    # block) — shipping it host-side avoids a slow GpSimd affine_select.
    wkq_d = nc.declare_dram_parameter("wkq", [KT, P, 2 * HEADS * DIM + P], BF16, isOutput=False)
    wvo_d = nc.declare_dram_parameter("wvo", [KT, P, 2 * HEADS * DIM], BF16, isOutput=False)
    b_d = nc.declare_dram_parameter("b", [DIM, 1], F32, isOutput=False)
    # y stored fp16: |y| <= ~0.4 here, fp16 adds ~5e-4 rel error but halves
    # the 4MB output DMA that dominated the kernel tail.
    y_d = nc.declare_dram_parameter("y", [DIM, N], F16, isOutput=True)

    OQT, OKT, OV, OOT = 0, HEADS * DIM, 0, HEADS * DIM

    prev_exit = tile.TileContext._drain_and_barrier
    tile.TileContext._drain_and_barrier = _minimal_exit
    try:
        _build_body(nc, tc_args=(x_d, xt_d, wkq_d, wvo_d, b_d, y_d, OQT, OKT, OV, OOT))
    finally:
        tile.TileContext._drain_and_barrier = prev_exit

    # NOTE: hoisting startup work before the init barrier was tried and lost
    # time — the runtime preamble (~6.5us) gates all engines anyway, and
    # pre-barrier work just delays the barrier release for everyone.
    _split_multi_waits(nc)
    return nc


def _build_body(nc, tc_args):
    x_d, xt_d, wkq_d, wvo_d, b_d, y_d, OQT, OKT, OV, OOT = tc_args
    with tile.TileContext(nc) as tc:
        with (
            tc.tile_pool(name="wpool", bufs=1) as wpool,
            tc.tile_pool(name="spool", bufs=2) as spool,
            tc.tile_pool(name="ypool", bufs=2) as ypool,
            tc.tile_pool(name="psum", bufs=1, space="PSUM") as psum,
        ):
            # ---- PE warmup: dummy matmuls during the input DMAs release
            # the HAM clock-gate; G's first xT slab completes ~10.3us
            # (per-queue bandwidth bound), so the warmup just covers the
            # wait.
            warm = wpool.tile([P, P], BF16, tag="warm")
            nc.gpsimd.memset(warm[:], 0)

            # warmup accumulates in the M banks (idle until ~22us) so G's
            # first matmul never waits on warmup retirement in its own bank.
            # 30 dummies cover the PE from the barrier (~7.8us) until xt
            # slab0 lands (~10.5us) with no idle gap, so the HAM clock-gate
            # flips to 8/8 at ~11us and G runs warm (2.4GHz) nearly
            # throughout; any idle gap resets the 3.4us HAM busy-window.
            wps = psum.tile([P, P], F32, tag="m", bufs=2)
            for _ in range(34):
                nc.tensor.matmul(wps[:], warm[:], warm[:], start=True, stop=True)

            # ---- loads (xT slabs first: G consumes them incrementally;
            # triggers split across the two HWDGE engines (SP + ACT) so the
            # trigger chains run in parallel) ----
            # xT slabs alternate the two HWDGE queues (Sync/Scalar).
            # NOTE: adding the GpSimd queue as a third input lane was tried
            # and lost time — qGpSimdDynamic moves bulk data at only
            # ~86 B/ns (half a HWDGE queue), so its slabs arrive LATE and
            # stall G. It is fine for the small y-phase store triggers.
            # NOTE: half-splitting slab0 for an earlier G start was tried —
            # the halves use 1KB DMA rows at ~half throughput, stretching
            # slab0 to ~3us, stalling G mid-stream and delaying every
            # later Sync-queue slab. Full 2KB-row slabs win.
            xt_sb = []
            for qi in range(NQ):
                t = wpool.tile([P, NT // NQ, DIM], F8, tag=f"xt{qi}")
                eng = nc.sync if qi % 2 == 0 else nc.scalar
                eng.dma_start(t[:], xt_d[qi])
                xt_sb.append(t)
            wkq_sb = []
            for k in range(KT):
                t = wpool.tile([P, 2 * HEADS * DIM + P], BF16, tag=f"wkq{k}")
                eng = nc.sync if k == 0 else nc.scalar
                eng.dma_start(t[:], wkq_d[k])
                wkq_sb.append(t)
            ident = wkq_sb[0][:, 2 * HEADS * DIM : 2 * HEADS * DIM + P]
            wvo_sb = []
            for k in range(KT):
                t = wpool.tile([P, 2 * HEADS * DIM], BF16, tag=f"wvo{k}")
                eng = nc.sync if k == 0 else nc.scalar
                eng.dma_start(t[:], wvo_d[k])
                wvo_sb.append(t)
            # dummy ACT op: forces the one-time ~1.3us activation-table
            # preload during the input DMAs instead of at the first real ACT
            # copy on the G->A critical path. Emitted AFTER the Scalar-queue
            # input triggers: ACT runs its trigger instructions inline, and
            # putting the table load first delayed Q10's first data ~1.3us.
            warm2 = wpool.tile([P, 1], BF16, tag="warm2")
            nc.scalar.copy(warm2[:], warm[:, 0:1])
            # x is shipped in [128, 2048] chunks (4KB DMA rows, full rate),
            # k0 chunks on the Sync ring, k1 chunks on the Scalar ring,
            # AFTER the xt/weight stream: the y GEMM consumes chunk c as
            # soon as both k-halves land instead of waiting for the whole
            # 2MB of x, and x never competes with the xt stream that paces
            # G. All triggers are emitted at load time: a trigger costs
            # ~0.6us of issuing-engine time, and emitting them late lets
            # the tile scheduler interleave them into ACT's softmax chain.
            XC = 2               # 2 chunks of 2048 per k-tile
            x_ch = [[wpool.tile([P, N // XC], BF16, tag=f"x{k}_{c}",
                                name=f"xsb{k}_{c}") for c in range(XC)]
                    for k in range(KT)]
            b_sb = []
            for ot in range(KT):
                b_sb.append(wpool.tile([P, 1], F32, tag=f"b{ot}", name=f"bsb{ot}"))
            # both k rows of x ride the Sync ring, interleaved so chunk
            # PAIRS complete together. Issuing any x triggers from ACT is
            # poison: the tile scheduler slots the flow-controlled ~0.6us
            # trigger instructions between ACT's softmax-chain ops.
            for c in range(XC):
                nc.sync.dma_start(x_ch[0][c][:], x_d[ts(0, P), ts(c, N // XC)])
                nc.sync.dma_start(x_ch[1][c][:], x_d[ts(1, P), ts(c, N // XC)])
            nc.sync.dma_start(b_sb[0][:], b_d[ts(0, P), :])
            nc.sync.dma_start(b_sb[1][:], b_d[ts(1, P), :])

            # ---- G = x @ x.T (fp32 PSUM, 32 accumulation steps) ----------
            # G is symmetric: compute row-tile 0 in full ([128, 256]) and
            # only the diagonal block of row-tile 1 ([128, 128], cols
            # 128:256); the (1,0) block is the PE-transposed (0,1) block.
            # Saves 32 N=128 matmuls (~1.7us warm) for one transpose+drain.
            g_ps = []
            for ct in range(KT):
                gp = psum.tile([P, DIM], F32, tag=f"g{ct}", bufs=1)
                g_ps.append(gp)
            for i in range(NT):
                qi, a = divmod(i, NT // NQ)
                nc.tensor.matmul(
                    g_ps[0][:],
                    xt_sb[qi][:, a, ts(0, P)],
                    xt_sb[qi][:, a, :],
                    start=(i == 0),
                    stop=(i == NT - 1),
                )
                nc.tensor.matmul(
                    g_ps[1][:, ts(1, P)],
                    xt_sb[qi][:, a, ts(1, P)],
                    xt_sb[qi][:, a, ts(1, P)],
                    start=(i == 0),
                    stop=(i == NT - 1),
                )
            g_sb = []
            for ct in range(KT):
                g = spool.tile([P, DIM], BF16, tag=f"gs{ct}", bufs=1, name=f"g{ct}")
                g_sb.append(g)
            # (0,1) block drains first on DVE: the transpose consumes it
            nc.vector.tensor_copy(g_sb[0][:, ts(1, P)], g_ps[0][:, ts(1, P)])
            nc.scalar.copy(g_sb[0][:, ts(0, P)], g_ps[0][:, ts(0, P)])
            nc.scalar.copy(g_sb[1][:, ts(1, P)], g_ps[1][:, ts(1, P)])
            # transpose lands in the first 'a'-rotation bank: its DVE drain
            # completes well before stage_A's second ct reuses that bank
            gtp = psum.tile([P, P], BF16, tag="a", bufs=2, name="gtp")
            nc.tensor.transpose(gtp[:], g_sb[0][:, ts(1, P)], ident)
            nc.vector.tensor_copy(g_sb[1][:, ts(0, P)], gtp[:])

            # ---- per-head stages, software-pipelined across heads --------
            # stage A(h): A = G @ Wk_h.T          (PE + drain)
            # stage L(h): L = (Wq_h*scale) @ A    (PE -> PSUM) + softmax
            # stage M(h): M_hT = Ehat . WoT       (PE + drain)
            # stage W(h): WstarT += Wv_h . M_hT   (PSUM accum in the G banks)
            a_all, es_all, lp_all = {}, {}, {}
            m_sb = {}
            wp_all = [
                psum.tile([P, DIM], F32, tag=f"g{ct}", bufs=1, name=f"wp{ct}")
                for ct in range(KT)
            ]
            wst_sb = []

            def stage_A(h):
                a_sb = []
                for ct in range(KT):
                    ap = psum.tile([P, DIM], F32, tag="a", bufs=2, name=f"ap{h}_{ct}")
                    for k in range(KT):
                        # A[c', d] = sum_c'' G[c'', c'] wkT[c'', d]  (G symmetric)
                        nc.tensor.matmul(
                            ap[:],
                            g_sb[k][:, ts(ct, P)],
                            wkq_sb[k][:, OKT + h * DIM : OKT + (h + 1) * DIM],
                            start=(k == 0),
                            stop=(k == KT - 1),
                        )
                    at = spool.tile([P, DIM], BF16, tag=f"a{ct}", name=f"at{h}_{ct}")
                    # all PSUM drains go to DVE: ACT is the head-phase
                    # pacer (exps + accumulator reads ~6.2us), while the
                    # softmax scale now runs on the idle Pool engine
                    nc.vector.tensor_copy(at[:], ap[:])
                    a_sb.append(at)
                a_all[h] = a_sb

            def stage_L(h):
                pl = []
                for ct in range(KT):
                    lp = psum.tile([P, DIM], F32, tag=f"l{ct}", bufs=1, name=f"lp{h}_{ct}")
                    for k in range(KT):
                        # L[c, d] = sum_c' wqT[c', c] A[c', d]
                        nc.tensor.matmul(
                            lp[:],
                            wkq_sb[k][:, OQT + h * DIM + ct * P : OQT + h * DIM + (ct + 1) * P],
                            a_all[h][k][:],
                            start=(k == 0),
                            stop=(k == KT - 1),
                        )
                    pl.append(lp)
                lp_all[h] = pl
                # softmax immediately (ACT/DVE; doesn't occupy the PE)
                es = []
                for ct in range(KT):
                    e = spool.tile([P, DIM], BF16, tag=f"e{ct}", name=f"e{h}_{ct}")
                    s = spool.tile([P, 1], F32, tag=f"s{ct}", name=f"s{h}_{ct}")
                    r = spool.tile([P, 1], F32, tag=f"r{ct}", name=f"r{h}_{ct}")
                    nc.scalar.activation(
                        e[:], pl[ct][:], mybir.ActivationFunctionType.Exp,
                        accum_out=s[:],
                    )
                    nc.vector.reciprocal(r[:], s[:])
                    # NOTE: Pool tensor_scalar was tried for this — 3880ns
                    # per [P,256] op (13x slower than DVE). Stay on DVE/ACT.
                    nc.any.tensor_scalar_mul(e[:], e[:], r[:])
                    es.append(e)
                es_all[h] = es

            def stage_M(h):
                es = es_all[h]
                for dt2 in range(KT):
                    pm = psum.tile([P, DIM], F32, tag="m", bufs=2, name=f"pm{h}_{dt2}")
                    for ct in range(KT):
                        # M_hT[d, o] = sum_c Ehat[c, d] woT[c, o]
                        nc.tensor.matmul(
                            pm[:],
                            es[ct][:, ts(dt2, P)],
                            wvo_sb[ct][:, OOT + h * DIM : OOT + (h + 1) * DIM],
                            start=(ct == 0),
                            stop=(ct == KT - 1),
                        )
                    mt = spool.tile([P, DIM], BF16, tag=f"m{h}_{dt2}", bufs=1,
                                    name=f"mt{h}_{dt2}")
                    m_sb[(h, dt2)] = mt
                    nc.vector.tensor_copy(mt[:], pm[:])

            def stage_W(h):
                # WstarT[c_in, o] += sum_d wv[d, c_in] M_hT[d, o]
                for ct in range(KT):
                    for dt2 in range(KT):
                        nc.tensor.matmul(
                            wp_all[ct][:],
                            wvo_sb[dt2][:, OV + h * DIM + ct * P : OV + h * DIM + (ct + 1) * P],
                            m_sb[(h, dt2)][:],
                            start=(h == 0 and dt2 == 0),
                            stop=(h == HEADS - 1 and dt2 == KT - 1),
                        )
                if h == HEADS - 1:
                    for ct in range(KT):
                        wt = spool.tile([P, DIM], BF16, tag=f"wst{ct}", bufs=1,
                                        name=f"wt{ct}")
                        # split by column half: the first y matmuls read
                        # only the ot0 halves, so draining those first
                        # starts the y phase earlier while the ot1 halves
                        # drain under the y GEMM. ct0 on DVE, ct1 on ACT
                        # (ACT's exps are done by now).
                        for half in range(KT):
                            src = wp_all[ct][:, ts(half, P)]
                            if ct == 0:
                                nc.vector.tensor_copy(wt[:, ts(half, P)], src)
                            else:
                                nc.scalar.copy(wt[:, ts(half, P)], src)
                        wst_sb.append(wt)

            # pipelined emission:
            # PE order A0 A1 L0 A2 L1 A3 L2 M0 L3 M1 W0 M2 W1 M3 W2 W3
            stage_A(0)
            stage_A(1)
            stage_L(0)
            stage_A(2)
            stage_L(1)
            stage_A(3)
            stage_L(2)
            stage_M(0)
            stage_L(3)
            stage_M(1)
            stage_W(0)
            stage_M(2)
            stage_W(1)
            stage_M(3)
            stage_W(2)
            stage_W(3)

            # ---- y = WstarT.T @ x + b ------------------------------------
            # drains alternate DVE/ACT writing fp16; chunks are DMA'd in
            # [128, 1024] j-pairs (2KB rows for full DMA descriptor
            # efficiency), trigger chains split across Sync/GpSimd queues
            y_sb = {}
            for ot in range(KT):
                y_sb[ot] = ypool.tile([P, N], F16, tag=f"y{ot}", bufs=1,
                                      name=f"ysb{ot}")
            ycnt = 0
            # 6-bank rotation: with only 4 banks the j+2'th matmul waits on
            # the j'th chunk's ~520ns drain, pacing the y GEMM at ~275ns/MM
            # instead of the 213ns warm matmul rate. The g banks are idle
            # once Wstar has drained, so they join the rotation (bufs=1
            # matches their earlier allocation).
            ytags = ["a", "a", "m", "m", "g0", "g1"]
            ybufs = {"a": 2, "m": 2, "g0": 1, "g1": 1}
            for j in range(NCH):
                for ot in range(KT):
                    ytag = ytags[ycnt % 6]
                    py = psum.tile([P, 512], F32, tag=ytag, bufs=ybufs[ytag],
                                   name=f"py{j}_{ot}")
                    for k in range(KT):
                        nc.tensor.matmul(
                            py[:],
                            wst_sb[k][:, ts(ot, P)],
                            x_ch[k][j // 4][:, ts(j % 4, 512)],
                            start=(k == 0),
                            stop=(k == KT - 1),
                        )
                    dst = y_sb[ot][:, ts(j, 512)]
                    if ycnt % 2 == 0:
                        nc.vector.tensor_scalar_add(dst, py[:], b_sb[ot][:])
                    else:
                        nc.scalar.add(dst, py[:], b_sb[ot][:])
                    ycnt += 1
                    # uniform j-pair stores: 2KB DMA rows run at ~2x the
                    # rate of 1KB-row singles, so a final 256KB pair beats
                    # a tapered 128KB single. All stores ride the Sync
                    # HWDGE ring (~300 B/ns solo, SP engine idle by now);
                    # the SWDGE ring only reached ~150 B/ns and its last
                    # transfer trailed the final matmul by ~4.6us. ACT as
                    # a lane was tried and lost 4us (trigger instructions
                    # land between ACT's y drains).
                    if j % 2 == 1:
                        nc.sync.dma_start(
                            y_d[ts(ot, P), (j - 1) * 512 : (j + 1) * 512],
                            y_sb[ot][:, (j - 1) * 512 : (j + 1) * 512],
                        )


def prep_inputs(x, w_qkv, w_out, b_out):
    """Host-side packing: per-core input dicts (numpy only)."""
    x = np.asarray(x, dtype=np.float32)
    w_qkv = np.asarray(w_qkv, dtype=np.float32)
    w_out = np.asarray(w_out, dtype=np.float32)
    b_out = np.asarray(b_out, dtype=np.float32)

    scale = float(DIM) ** -0.5
    wq = w_qkv[0 * HEADS * DIM : 1 * HEADS * DIM].reshape(HEADS, DIM, DIM)
    wk = w_qkv[1 * HEADS * DIM : 2 * HEADS * DIM].reshape(HEADS, DIM, DIM)
    wv = w_qkv[2 * HEADS * DIM : 3 * HEADS * DIM].reshape(HEADS, DIM, DIM)

    # wqT[c', h*256 + c] = wq[h, c, c'] * scale * XT_SCALE^2 (undoes the
    # fp8 xt downscale baked into G)
    wqT = (np.transpose(wq, (2, 0, 1)) * (scale * XT_SCALE * XT_SCALE)).reshape(
        DIM, HEADS * DIM)
    # wkT[c', h*256 + d] = wk[h, d, c']
    wkT = np.transpose(wk, (2, 0, 1)).reshape(DIM, HEADS * DIM)
    # wvn[d, h*256 + c_in] = wv[h, d, c_in]  (natural orientation, head-concat)
    wvn = np.transpose(wv, (1, 0, 2)).reshape(DIM, HEADS * DIM)
    # woT[c, h*256 + o] = w_out[o, c*HEADS + h]
    woT = np.ascontiguousarray(
        w_out.reshape(DIM, DIM, HEADS).transpose(1, 2, 0)
    ).reshape(DIM, HEADS * DIM)

    # wkq[k] = [wqT | wkT | I/0], wvo[k] = [wv | woT], rows k*128:(k+1)*128
    id_col = np.concatenate([np.eye(P, dtype=np.float32),
                             np.zeros((P, P), dtype=np.float32)], axis=0)
    wkq = np.ascontiguousarray(
        np.concatenate([wqT, wkT, id_col], axis=1).astype(NPBF16)
        .reshape(KT, P, 2 * HEADS * DIM + P)
    )
    wvo = np.ascontiguousarray(
        np.concatenate([wvn, woT], axis=1).astype(NPBF16).reshape(KT, P, 2 * HEADS * DIM)
    )
    b = b_out.reshape(DIM, 1).astype(np.float32)

    in_maps = []
    for bi in range(B):
        xbf = np.ascontiguousarray(x[bi].reshape(DIM, N))
        xb = xbf.astype(NPBF16)
        # xt[qi, p, a, c] = x.T[qi*(N//NQ) + a*128 + p, c] / XT_SCALE, fp8
        xt = np.ascontiguousarray(
            (xbf.T / XT_SCALE).astype(NPF8).reshape(NQ, NT // NQ, P, DIM)
            .transpose(0, 2, 1, 3)
        )
        in_maps.append({"x": xb, "xt": xt, "wkq": wkq, "wvo": wvo, "b": b})
    return in_maps


_NC_CACHE = {}


def get_program():
    if "nc" not in _NC_CACHE:
        _NC_CACHE["nc"] = build_program()
    return _NC_CACHE["nc"]


def kernel(x, w_qkv, w_out, b_out, **_unused):
    nc = get_program()
    in_maps = prep_inputs(x, w_qkv, w_out, b_out)
    res = run_bass_kernel_spmd(nc, in_maps, list(range(N_CORES)))
    y = np.stack([res.results[c]["y"] for c in range(N_CORES)], axis=0)
    return y.reshape(B, DIM, H, W).astype(np.float32)

